# revision 14
# baseline (speedup 1.0000x reference)
"""Trainium2 Bass kernel for CausalSelfAttentionWithCache (sink+window KV cache).

Sharding: 8 cores = batch(2) x head-group(4). Each core owns one batch element
and 4 heads (512 channels): Megatron column-parallel Wq/Wk/Wv, row-parallel
Wproj (the 4 head-group partial outputs are summed on the host). QK-LayerNorm
is over the full 2048 channels, so per-token sum / sum-of-squares are
AllReduced on-device across the 4 cores of each batch group (8-core AllReduce,
batch selected by a per-core mask input; <=4-core shared-output collectives
are unsupported, and compute engines can only address partition bases
{0,32,64}, so batch slots live in the free dimension).

Device pipeline per core:
  A) q/k/v projections in natural [token, channel] layout (x^T chunks
     stationary, W^T moving, fp32r), per-token LN stats via free-dim reduces
     (ScalarE Square+accum / VectorE reduce), tiny [128,72] AllReduce, LN
     apply + partial 2D-axial RoPE on VectorE, then PE transposes into the
     q^T / k^T (d, t) layout that attention needs.
  B) attention per (head, t-half): S^T = K^T-chunk-stationary @ q^T-moving
     (fp32r), exp on ScalarE with the 1/sqrt(128) scale fused (no max
     subtraction -- logits are O(1) by construction), P^T bf16 resident in
     SBUF; AV and the softmax denominator as V-/ones-stationary matmuls over
     the same P^T moving stream, PSUM-accumulated over l-chunks; normalized
     with VectorE reciprocal. The attention output O^T doubles as y^T.
  C) out-projection: y^T chunks stationary, Wproj^T moving, fp32r; partial
     [1035, 2048] DMA'd out per core.
"""

import math
import os
from functools import lru_cache

import ml_dtypes
import numpy as np

# ---- problem constants ----
N_HEAD = 16
HEAD_DIM = 128
N_EMBD = 2048
PREFIX = 11
PATCH_X, PATCH_Y = 32, 32
T_TOK = PREFIX + PATCH_X * PATCH_Y  # 1035
SINK_TOKENS = T_TOK
MAX_ATTN = 4 * T_TOK  # 4140
THETA = 1000.0
EPS = 1e-5

N_CORES = 8
HPG = 4                      # heads per group (core)
CPG = HPG * HEAD_DIM         # 512 channels per core
NCC = N_EMBD // 128          # 16 contraction chunks

LAST_EXEC_NS = None  # set when TRN_KERNEL_TRACE=1


def _chunks(total, step=128):
    out = []
    o = 0
    while o < total:
        out.append((o, min(step, total - o)))
        o += step
    return out


def _rope_tables():
    """cosN/sinN [T_TOK, CPG] fp32, natural (token, channel) layout.

    Column ch: pair index i = (ch % 128)//2 (same for every head). t < PREFIX:
    identity (cos=1, sin=0). sinN sign baked in: even cols -sin, odd cols +sin,
    pairing with an adjacent-channel swap to realize the 2x2 rotation.
    """
    dim = HEAD_DIM
    n = dim // 4
    freqs = 1.0 / (THETA ** (np.arange(0, dim, 4)[:n].astype(np.float32) / dim))
    t = np.arange(PATCH_X * PATCH_Y, dtype=np.float32)
    tx = t % PATCH_X
    ty = np.floor(t / PATCH_X)
    ang = np.concatenate([np.outer(tx, freqs), np.outer(ty, freqs)], axis=-1)
    cos = np.cos(ang)  # [1024, 64]
    sin = np.sin(ang)
    cosN = np.ones((T_TOK, CPG), np.float32)
    sinN = np.zeros((T_TOK, CPG), np.float32)
    ch = np.arange(CPG)
    pair = (ch % HEAD_DIM) // 2
    cosN[PREFIX:, :] = cos[:, pair]
    sign = np.where(ch % 2 == 0, -1.0, 1.0).astype(np.float32)[None, :]
    sinN[PREFIX:, :] = sin[:, pair] * sign
    return cosN, sinN


def _split_excess_waits(nc, mybir, max_waits=1):
    """This walrus build caps semaphore waits per CTRL instruction. Move the
    excess onto preceding same-engine NoOps (semantically identical)."""
    ctr = 0
    for f in nc.m.functions:
        for bb in f.blocks:
            insts = bb.instructions
            if not any(
                i.sync_info is not None
                and i.sync_info.on_wait
                and len(i.sync_info.on_wait) > max_waits
                for i in insts
            ):
                continue
            new_list = []
            for inst in insts:
                si = inst.sync_info
                if si is not None and si.on_wait and len(si.on_wait) > max_waits:
                    waits = list(si.on_wait)
                    extra, keep = waits[:-max_waits], waits[-max_waits:]
                    for j in range(0, len(extra), max_waits):
                        ctr += 1
                        nop = mybir.InstNoOp(name=f"I-WSPLIT-{ctr}", ins=[], outs=[])
                        nop.engine = inst.engine
                        nop.sync_info = mybir.SyncInfo(
                            on_wait=extra[j : j + max_waits], on_update=[]
                        )
                        new_list.append(nop)
                    inst.sync_info = mybir.SyncInfo(
                        on_wait=keep, on_update=list(si.on_update or [])
                    )
                new_list.append(inst)
            bb.instructions[:] = new_list


@lru_cache(maxsize=2)
def _build(L_cache):
    import concourse.bass as bass
    import concourse.mybir as mybir
    import concourse.tile as tile
    from concourse.masks import make_identity

    f32 = mybir.dt.float32
    f32r = mybir.dt.float32r
    bf16 = mybir.dt.bfloat16
    AF = mybir.ActivationFunctionType
    ALU = mybir.AluOpType
    AX = mybir.AxisListType

    T = T_TOK
    L = L_cache + T
    SCALE = 1.0 / math.sqrt(HEAD_DIM)

    cache_ch = [(o, s) for (o, s) in _chunks(L_cache)]
    TCH = _chunks(T)
    NTC = len(TCH)
    LCH = cache_ch + [(L_cache + o, s) for (o, s) in TCH]
    NLC = len(LCH)
    NVC = len(cache_ch)
    T_PAD = T + (T % 2)  # fp32r matmuls need an even moving size
    HALVES = [(0, 512), (512, T - 512)]
    HMAX = max(s + s % 2 for _, s in HALVES)
    NST = 2 * NTC * 4  # AllReduce staging width (both batch slots)

    nc = bass.Bass(num_devices=N_CORES)

    xT = nc.declare_dram_parameter("xT", [N_EMBD, T], f32r, isOutput=False)
    wqT = nc.declare_dram_parameter("wqT", [N_EMBD, CPG], f32r, isOutput=False)
    wkT = nc.declare_dram_parameter("wkT", [N_EMBD, CPG], f32r, isOutput=False)
    wvT = nc.declare_dram_parameter("wvT", [N_EMBD, CPG], f32r, isOutput=False)
    wpT = nc.declare_dram_parameter("wpT", [CPG, N_EMBD], f32r, isOutput=False)
    kTc = nc.declare_dram_parameter(
        "kTc", [HPG, HEAD_DIM, max(L_cache, 1)], f32r, isOutput=False
    )
    vc = nc.declare_dram_parameter(
        "vc", [HPG, max(L_cache, 1), HEAD_DIM], bf16, isOutput=False
    )
    cosN = nc.declare_dram_parameter("cosN", [T, CPG], f32, isOutput=False)
    sinN = nc.declare_dram_parameter("sinN", [T, CPG], f32, isOutput=False)
    qn_w = nc.declare_dram_parameter("qn_w", [CPG], f32, isOutput=False)
    qn_b = nc.declare_dram_parameter("qn_b", [CPG], f32, isOutput=False)
    kn_w = nc.declare_dram_parameter("kn_w", [CPG], f32, isOutput=False)
    kn_b = nc.declare_dram_parameter("kn_b", [CPG], f32, isOutput=False)
    bmask = nc.declare_dram_parameter("bmask", [128, 2], f32, isOutput=False)
    outp = nc.declare_dram_parameter("outp", [T, N_EMBD], f32, isOutput=True)

    ar_in = nc.dram_tensor("ar_in", [128, NST], f32)
    ar_out = nc.dram_tensor("ar_out", [128, NST], f32, addr_space="Shared")

    with tile.TileContext(nc) as tc:
        with tc.tile_pool(name="persist", bufs=1) as p1:
            qT = p1.tile([128, HPG, T_PAD], f32r)
            if T_PAD > T:
                nc.vector.memset(qT[:, :, T:T_PAD].bitcast(f32), 0.0)
            kN = p1.tile([128, HPG, T], f32r)
            vN = p1.tile([128, HPG, NTC, HEAD_DIM], bf16)
            ones_bf = p1.tile([128, 128], bf16)
            nc.vector.memset(ones_bf, 1.0)
            ident = p1.tile([128, 128], f32)
            make_identity(nc, ident)

            # ============ phase A: projections + stats + LN + rope ==========
            with tc.tile_pool(name="ln", bufs=1) as pln:
                z_sb = pln.tile([128, 2, NTC, CPG], f32)
                stl = pln.tile([128, NTC, 4], f32)  # qsum, qsq, ksum, ksq
                stg = pln.tile([128, NTC, 4], f32)
                stage = pln.tile([128, NST], f32)
                garp = pln.tile([128, NST], f32)
                t36 = pln.tile([128, NTC * 4], f32)
                mu2 = pln.tile([128, 2, NTC], f32)
                r2 = pln.tile([128, 2, NTC], f32)
                m2 = pln.tile([128, 2, NTC], f32)
                sc1 = pln.tile([128, NTC], f32)
                sc2 = pln.tile([128, NTC], f32)
                sc3 = pln.tile([128, NTC], f32)
                epsT = pln.tile([128, 1], f32)
                nc.vector.memset(epsT, EPS)
                bmt = pln.tile([128, 2], f32)
                nc.sync.dma_start(out=bmt, in_=bmask[:, :])
                wbb = pln.tile([128, 2, 2, CPG], f32)  # (q/k, w/b) bcast
                for i, src in enumerate((qn_w, qn_b, kn_w, kn_b)):
                    sap = src[:]
                    nc.sync.dma_start(
                        out=wbb[:, i // 2, i % 2, :],
                        in_=bass.AP(
                            tensor=sap.tensor,
                            offset=sap.offset,
                            ap=[[0, 128]] + list(sap.ap),
                        ),
                    )
                nc.vector.memset(stl, 0.0)

                with tc.tile_pool(name="proj", bufs=1) as pa, \
                     tc.tile_pool(name="sqs", bufs=2) as psq, \
                     tc.tile_pool(name="ps_z", bufs=3, space="PSUM") as ps_z:
                    xs = pa.tile([128, NCC, T], f32r)
                    nc.sync.dma_start(
                        out=xs, in_=xT.rearrange("(c p) t -> p c t", p=128)
                    )

                    # --- q, k projections + per-token stats ---
                    for pi, wsrc in enumerate((wqT, wkT)):
                        w_t = pa.tile([128, NCC, CPG], f32r, tag="w")
                        nc.sync.dma_start(
                            out=w_t, in_=wsrc.rearrange("(c p) n -> p c n", p=128)
                        )
                        for tci, (t0, tsz) in enumerate(TCH):
                            zp = ps_z.tile([128, CPG], f32, tag="zp")
                            for cc in range(NCC):
                                nc.tensor.matmul(
                                    zp[:tsz, :],
                                    lhsT=xs[:, cc, t0 : t0 + tsz],
                                    rhs=w_t[:, cc, :],
                                    start=(cc == 0),
                                    stop=(cc == NCC - 1),
                                )
                            dst = z_sb[:tsz, pi, tci, :]
                            nc.vector.tensor_copy(dst, zp[:tsz, :])
                            sq = psq.tile([128, CPG], f32, tag="sq")
                            nc.scalar.activation(
                                sq[:tsz, :],
                                zp[:tsz, :],
                                AF.Square,
                                accum_out=stl[:tsz, tci, 2 * pi + 1 : 2 * pi + 2],
                            )
                            nc.vector.tensor_reduce(
                                stl[:tsz, tci, 2 * pi : 2 * pi + 1],
                                dst,
                                axis=AX.X,
                                op=ALU.add,
                            )

                    # --- masked 8-core AllReduce of the stats ---
                    stl_f = stl.rearrange("p a b -> p (a b)")
                    nc.vector.tensor_scalar(
                        stage[:, 0 : NST // 2], stl_f, bmt[:, 0:1], None, op0=ALU.mult
                    )
                    nc.vector.tensor_scalar(
                        stage[:, NST // 2 : NST], stl_f, bmt[:, 1:2], None,
                        op0=ALU.mult,
                    )
                    nc.sync.dma_start(out=ar_in[:, :], in_=stage)
                    nc.gpsimd.collective_compute(
                        "AllReduce",
                        ALU.add,
                        replica_groups=[[0, 1, 2, 3, 4, 5, 6, 7]],
                        ins=[ar_in[:, :]],
                        outs=[ar_out[:, :]],
                    )
                    garp_ = garp
                    nc.sync.dma_start(out=garp_, in_=ar_out[:, :])
                    stg_f = stg.rearrange("p a b -> p (a b)")
                    nc.vector.tensor_scalar(
                        t36, garp_[:, 0 : NST // 2], bmt[:, 0:1], None, op0=ALU.mult
                    )
                    nc.vector.tensor_scalar(
                        stg_f, garp_[:, NST // 2 : NST], bmt[:, 1:2], None,
                        op0=ALU.mult,
                    )
                    nc.vector.tensor_add(stg_f, stg_f, t36)

                    # --- v projection (overlaps the collective) ---
                    with tc.tile_pool(name="ps_v", bufs=2, space="PSUM") as ps_v:
                        wv_t = pa.tile([128, NCC, CPG], f32r, tag="w")
                        nc.sync.dma_start(
                            out=wv_t, in_=wvT.rearrange("(c p) n -> p c n", p=128)
                        )
                        for tci, (t0, tsz) in enumerate(TCH):
                            vp = ps_v.tile([128, CPG], f32)
                            for cc in range(NCC):
                                nc.tensor.matmul(
                                    vp[:tsz, :],
                                    lhsT=xs[:, cc, t0 : t0 + tsz],
                                    rhs=wv_t[:, cc, :],
                                    start=(cc == 0),
                                    stop=(cc == NCC - 1),
                                )
                            nc.vector.tensor_copy(
                                vN[:tsz, :, tci, :],
                                vp[:tsz, :].rearrange("p (h d) -> p h d", h=HPG),
                            )

                    # --- LN scalar math: mu, rstd, mu*rstd  [128, NTC] each ---
                    for pi in range(2):
                        nc.vector.tensor_scalar(
                            mu2[:, pi, :], stg[:, :, 2 * pi], 1.0 / N_EMBD, None,
                            op0=ALU.mult,
                        )
                        nc.vector.tensor_scalar(
                            sc1, stg[:, :, 2 * pi + 1], 1.0 / N_EMBD, None,
                            op0=ALU.mult,
                        )
                        nc.vector.tensor_mul(sc2, mu2[:, pi, :], mu2[:, pi, :])
                        nc.vector.tensor_sub(sc3, sc1, sc2)
                        nc.scalar.activation(sc1, sc3, AF.Sqrt, bias=epsT)
                        nc.vector.reciprocal(r2[:, pi, :], sc1)
                        nc.vector.tensor_mul(m2[:, pi, :], mu2[:, pi, :], r2[:, pi, :])

                # --- LN apply + rope + transpose to (d, t) layout ---
                with tc.tile_pool(name="rope", bufs=2) as pr, \
                     tc.tile_pool(name="rtmp", bufs=2) as prt, \
                     tc.tile_pool(name="ps_t", bufs=4, space="PSUM") as ps_t:
                    for tci, (t0, tsz) in enumerate(TCH):
                        cs = pr.tile([128, CPG], f32, tag="cos")
                        nc.sync.dma_start(out=cs[:tsz, :], in_=cosN[t0 : t0 + tsz, :])
                        sn = pr.tile([128, CPG], f32, tag="sin")
                        nc.sync.dma_start(out=sn[:tsz, :], in_=sinN[t0 : t0 + tsz, :])
                        for pi in range(2):
                            z = z_sb[:tsz, pi, tci, :]
                            t1 = prt.tile([128, CPG], f32, tag="A")
                            nc.vector.tensor_scalar(
                                t1[:tsz, :],
                                z,
                                mu2[:tsz, pi, tci : tci + 1],
                                r2[:tsz, pi, tci : tci + 1],
                                op0=ALU.subtract,
                                op1=ALU.mult,
                            )
                            t2 = prt.tile([128, CPG], f32, tag="B")
                            nc.vector.tensor_mul(
                                t2[:tsz, :], t1[:tsz, :], wbb[:tsz, pi, 0, :]
                            )
                            t3 = prt.tile([128, CPG], f32, tag="A")
                            nc.vector.tensor_add(
                                t3[:tsz, :], t2[:tsz, :], wbb[:tsz, pi, 1, :]
                            )
                            sw = prt.tile([128, CPG], f32, tag="B")
                            swr = sw.rearrange("p (i two) -> p i two", two=2)
                            t3r = t3.rearrange("p (i two) -> p i two", two=2)
                            nc.vector.tensor_copy(
                                swr[:tsz, :, 0:1], t3r[:tsz, :, 1:2]
                            )
                            nc.vector.tensor_copy(
                                swr[:tsz, :, 1:2], t3r[:tsz, :, 0:1]
                            )
                            a = prt.tile([128, CPG], f32, tag="A2")
                            nc.vector.tensor_mul(a[:tsz, :], t3[:tsz, :], cs[:tsz, :])
                            b = prt.tile([128, CPG], f32, tag="B2")
                            nc.vector.tensor_mul(b[:tsz, :], sw[:tsz, :], sn[:tsz, :])
                            r = prt.tile([128, CPG], f32, tag="C")
                            nc.vector.tensor_add(r[:tsz, :], a[:tsz, :], b[:tsz, :])
                            dstq = (qT if pi == 0 else kN)
                            for ci in range(HPG):
                                tp = ps_t.tile([128, 128], f32)
                                nc.tensor.transpose(
                                    tp[:, :tsz],
                                    r[:tsz, ci * 128 : (ci + 1) * 128],
                                    ident[:tsz, :tsz],
                                )
                                nc.vector.tensor_copy(
                                    dstq[:, ci, t0 : t0 + tsz], tp[:, :tsz]
                                )

            # ================= phase B: attention ===========================
            with tc.tile_pool(name="yt", bufs=1) as py:
                yT = py.tile([128, HPG, T], f32r)
                with tc.tile_pool(name="kt", bufs=2) as pb_kt, \
                     tc.tile_pool(name="vv", bufs=2) as pb_v, \
                     tc.tile_pool(name="pt", bufs=2) as pb_pt, \
                     tc.tile_pool(name="rcp", bufs=2) as pb_rc, \
                     tc.tile_pool(name="ps_s", bufs=2, space="PSUM") as ps_s, \
                     tc.tile_pool(name="ps_av", bufs=1, space="PSUM") as ps_av, \
                     tc.tile_pool(name="ps_dn", bufs=1, space="PSUM") as ps_dn:
                    for h in range(HPG):
                        KT_h = pb_kt.tile([128, L], f32r)
                        if L_cache > 0:
                            nc.sync.dma_start(out=KT_h[:, :L_cache], in_=kTc[h])
                        nc.vector.tensor_copy(KT_h[:, L_cache:], kN[:, h, :])
                        V_h = pb_v.tile([128, NLC, HEAD_DIM], bf16)
                        ncf, rem = L_cache // 128, L_cache % 128
                        if ncf:
                            nc.sync.dma_start(
                                out=V_h[:, 0:ncf, :],
                                in_=vc[h, 0 : ncf * 128, :].rearrange(
                                    "(c p) d -> p c d", p=128
                                ),
                            )
                        if rem:
                            nc.sync.dma_start(
                                out=V_h[0:rem, ncf, :],
                                in_=vc[h, ncf * 128 : L_cache, :],
                            )
                        for tci, (t0, tsz) in enumerate(TCH):
                            nc.vector.tensor_copy(
                                V_h[:tsz, NVC + tci, :], vN[:tsz, h, tci, :]
                            )
                        for (t0, tsz) in HALVES:
                            PT = pb_pt.tile([128, NLC, HMAX], bf16)
                            for li, (l0, lsz) in enumerate(LCH):
                                sp = ps_s.tile([128, HMAX], f32)
                                for (m0, msz) in _chunks(tsz + tsz % 2, 512):
                                    nc.tensor.matmul(
                                        sp[0:lsz, m0 : m0 + msz],
                                        lhsT=KT_h[:, l0 : l0 + lsz],
                                        rhs=qT[
                                            :, h, t0 + m0 : t0 + m0 + msz
                                        ],
                                        start=True,
                                        stop=True,
                                    )
                                nc.scalar.activation(
                                    PT[0:lsz, li, 0:tsz],
                                    sp[0:lsz, 0:tsz],
                                    AF.Exp,
                                    scale=SCALE,
                                )
                            av = ps_av.tile([128, HMAX], f32)
                            den = ps_dn.tile([128, HMAX], f32)
                            for li, (l0, lsz) in enumerate(LCH):
                                st, sp_ = (li == 0), (li == NLC - 1)
                                for (m0, msz) in _chunks(tsz, 512):
                                    nc.tensor.matmul(
                                        av[:, m0 : m0 + msz],
                                        lhsT=V_h[0:lsz, li, :],
                                        rhs=PT[0:lsz, li, m0 : m0 + msz],
                                        start=st,
                                        stop=sp_,
                                    )
                                    nc.tensor.matmul(
                                        den[:, m0 : m0 + msz],
                                        lhsT=ones_bf[0:lsz, :],
                                        rhs=PT[0:lsz, li, m0 : m0 + msz],
                                        start=st,
                                        stop=sp_,
                                    )
                            rc = pb_rc.tile([128, HMAX], f32)
                            nc.vector.reciprocal(rc[:, 0:tsz], den[:, 0:tsz])
                            nc.vector.tensor_mul(
                                yT[:, h, t0 : t0 + tsz], av[:, 0:tsz], rc[:, 0:tsz]
                            )

                # ================= phase C: out-projection ==================
                with tc.tile_pool(name="wp", bufs=1) as pwp, \
                     tc.tile_pool(name="oc", bufs=2) as poc, \
                     tc.tile_pool(name="ps_o", bufs=2, space="PSUM") as ps_o:
                    wp_sb = pwp.tile([128, HPG, N_EMBD], f32r)
                    nc.sync.dma_start(
                        out=wp_sb, in_=wpT.rearrange("(h p) n -> p h n", p=128)
                    )
                    for (t0, tsz) in TCH:
                        op = ps_o.tile([128, N_EMBD], f32)
                        for co in range(4):
                            for h in range(HPG):
                                nc.tensor.matmul(
                                    op[0:tsz, co * 512 : (co + 1) * 512],
                                    lhsT=yT[:, h, t0 : t0 + tsz],
                                    rhs=wp_sb[:, h, co * 512 : (co + 1) * 512].bitcast(
                                        f32r
                                    ),
                                    start=(h == 0),
                                    stop=(h == HPG - 1),
                                )
                        ot = poc.tile([128, N_EMBD], f32)
                        nc.vector.tensor_copy(ot[0:tsz, :], op[0:tsz, :])
                        nc.sync.dma_start(out=outp[t0 : t0 + tsz, :], in_=ot[0:tsz, :])

    import concourse.mybir as mybir_mod

    _split_excess_waits(nc, mybir_mod, max_waits=1)
    return nc


def _window_rows(T, cache_size, current_start, global_end_index, local_end_index):
    """Replicates the reference's sink+sliding-window eviction arithmetic;
    returns the cache row indices of the attention window (new rows follow)."""
    current_end = current_start + T
    if current_end > global_end_index and T + local_end_index > cache_size:
        num_evicted = T + local_end_index - cache_size
        num_rolled = local_end_index - num_evicted - SINK_TOKENS
        new_local_end = (
            local_end_index + current_end - global_end_index - num_evicted
        )
        cache_rows = list(range(SINK_TOKENS)) + list(
            range(SINK_TOKENS + num_evicted, SINK_TOKENS + num_evicted + num_rolled)
        )
    else:
        new_local_end = local_end_index + current_end - global_end_index
        cache_rows = list(range(new_local_end - T))
    cache_start = max(0, new_local_end - MAX_ATTN)
    m = len(cache_rows)
    assert cache_start <= m and new_local_end - m == T, (
        "kernel supports windows that contain all new tokens"
    )
    return cache_rows[cache_start:m]


def kernel(**inputs):
    global LAST_EXEC_NS
    from concourse.bass_utils import run_bass_kernel_spmd

    x = np.asarray(inputs["x"], np.float32)
    Wq = np.asarray(inputs["Wq"], np.float32)
    Wk = np.asarray(inputs["Wk"], np.float32)
    Wv = np.asarray(inputs["Wv"], np.float32)
    Wproj = np.asarray(inputs["Wproj"], np.float32)
    qn_w = np.asarray(inputs["qn_w"], np.float32)
    qn_b = np.asarray(inputs["qn_b"], np.float32)
    kn_w = np.asarray(inputs["kn_w"], np.float32)
    kn_b = np.asarray(inputs["kn_b"], np.float32)
    cache_k = np.asarray(inputs["cache_k"], np.float32)
    cache_v = np.asarray(inputs["cache_v"], np.float32)
    cs = int(inputs["current_start"])
    ge = int(inputs["global_end_index"])
    le = int(inputs["local_end_index"])

    Bsz, T, C = x.shape
    assert (Bsz, T, C) == (2, T_TOK, N_EMBD)
    win = np.asarray(
        _window_rows(T, cache_k.shape[1], cs, ge, le), dtype=np.int64
    )
    L_cache = len(win)

    nc = _build(L_cache)

    cosN, sinN = _rope_tables()
    in_maps = []
    for core in range(N_CORES):
        b, hg = core // 4, core % 4
        ch0, hs = hg * CPG, hg * HPG
        kc = (
            cache_k[b][win][:, hs : hs + HPG, :]
            if L_cache
            else np.zeros((1, HPG, HEAD_DIM), np.float32)
        )
        vv = (
            cache_v[b][win][:, hs : hs + HPG, :]
            if L_cache
            else np.zeros((1, HPG, HEAD_DIM), np.float32)
        )
        bm = np.zeros((128, 2), np.float32)
        bm[:, b] = 1.0
        in_maps.append(
            {
                "xT": np.ascontiguousarray(x[b].T),
                "wqT": np.ascontiguousarray(Wq[ch0 : ch0 + CPG, :].T),
                "wkT": np.ascontiguousarray(Wk[ch0 : ch0 + CPG, :].T),
                "wvT": np.ascontiguousarray(Wv[ch0 : ch0 + CPG, :].T),
                "wpT": np.ascontiguousarray(Wproj[:, ch0 : ch0 + CPG].T),
                "kTc": np.ascontiguousarray(kc.transpose(1, 2, 0)),
                "vc": np.ascontiguousarray(vv.transpose(1, 0, 2)).astype(
                    ml_dtypes.bfloat16
                ),
                "cosN": cosN,
                "sinN": sinN,
                "qn_w": np.ascontiguousarray(qn_w[ch0 : ch0 + CPG]),
                "qn_b": np.ascontiguousarray(qn_b[ch0 : ch0 + CPG]),
                "kn_w": np.ascontiguousarray(kn_w[ch0 : ch0 + CPG]),
                "kn_b": np.ascontiguousarray(kn_b[ch0 : ch0 + CPG]),
                "bmask": bm,
            }
        )

    trace = os.environ.get("TRN_KERNEL_TRACE", "0") == "1"
    res = run_bass_kernel_spmd(
        nc, in_maps, core_ids=list(range(N_CORES)), trace=trace
    )
    if trace:
        LAST_EXEC_NS = res.exec_time_ns

    out = np.zeros((Bsz, T, C), np.float32)
    for core in range(N_CORES):
        out[core // 4] += res.results[core]["outp"]
    return out


# revision 16
# speedup vs baseline: 1.0053x; 1.0053x over previous
"""Trainium2 Bass kernel for CausalSelfAttentionWithCache (sink+window KV cache).

Sharding: 8 cores = batch(2) x head-group(4). Each core owns one batch element
and 4 heads (512 channels): Megatron column-parallel Wq/Wk/Wv, row-parallel
Wproj (the 4 head-group partial outputs are summed on the host). QK-LayerNorm
is over the full 2048 channels, so per-token sum / sum-of-squares are
AllReduced on-device across the 4 cores of each batch group (8-core AllReduce,
batch selected by a per-core mask input; <=4-core shared-output collectives
are unsupported, and compute engines can only address partition bases
{0,32,64}, so batch slots live in the free dimension).

Device pipeline per core:
  A) q/k/v projections in natural [token, channel] layout (x^T chunks
     stationary, W^T moving, fp32r), per-token LN stats via free-dim reduces
     (ScalarE Square+accum / VectorE reduce), tiny [128,72] AllReduce, LN
     apply + partial 2D-axial RoPE on VectorE, then PE transposes into the
     q^T / k^T (d, t) layout that attention needs.
  B) attention per (head, t-half): S^T = K^T-chunk-stationary @ q^T-moving
     (fp32r), exp on ScalarE with the 1/sqrt(128) scale fused (no max
     subtraction -- logits are O(1) by construction), P^T bf16 resident in
     SBUF; AV and the softmax denominator as V-/ones-stationary matmuls over
     the same P^T moving stream, PSUM-accumulated over l-chunks; normalized
     with VectorE reciprocal. The attention output O^T doubles as y^T.
  C) out-projection: y^T chunks stationary, Wproj^T moving, fp32r; partial
     [1035, 2048] DMA'd out per core.
"""

import math
import os
from functools import lru_cache

import ml_dtypes
import numpy as np

# ---- problem constants ----
N_HEAD = 16
HEAD_DIM = 128
N_EMBD = 2048
PREFIX = 11
PATCH_X, PATCH_Y = 32, 32
T_TOK = PREFIX + PATCH_X * PATCH_Y  # 1035
SINK_TOKENS = T_TOK
MAX_ATTN = 4 * T_TOK  # 4140
THETA = 1000.0
EPS = 1e-5

N_CORES = 8
HPG = 4                      # heads per group (core)
CPG = HPG * HEAD_DIM         # 512 channels per core
NCC = N_EMBD // 128          # 16 contraction chunks

LAST_EXEC_NS = None  # set when TRN_KERNEL_TRACE=1


def _chunks(total, step=128):
    out = []
    o = 0
    while o < total:
        out.append((o, min(step, total - o)))
        o += step
    return out


def _rope_tables():
    """cosN/sinN [T_TOK, CPG] fp32, natural (token, channel) layout.

    Column ch: pair index i = (ch % 128)//2 (same for every head). t < PREFIX:
    identity (cos=1, sin=0). sinN sign baked in: even cols -sin, odd cols +sin,
    pairing with an adjacent-channel swap to realize the 2x2 rotation.
    """
    dim = HEAD_DIM
    n = dim // 4
    freqs = 1.0 / (THETA ** (np.arange(0, dim, 4)[:n].astype(np.float32) / dim))
    t = np.arange(PATCH_X * PATCH_Y, dtype=np.float32)
    tx = t % PATCH_X
    ty = np.floor(t / PATCH_X)
    ang = np.concatenate([np.outer(tx, freqs), np.outer(ty, freqs)], axis=-1)
    cos = np.cos(ang)  # [1024, 64]
    sin = np.sin(ang)
    cosN = np.ones((T_TOK, CPG), np.float32)
    sinN = np.zeros((T_TOK, CPG), np.float32)
    ch = np.arange(CPG)
    pair = (ch % HEAD_DIM) // 2
    cosN[PREFIX:, :] = cos[:, pair]
    sign = np.where(ch % 2 == 0, -1.0, 1.0).astype(np.float32)[None, :]
    sinN[PREFIX:, :] = sin[:, pair] * sign
    return cosN, sinN


def _split_excess_waits(nc, mybir, max_waits=1):
    """This walrus build caps semaphore waits per CTRL instruction. Move the
    excess onto preceding same-engine NoOps (semantically identical)."""
    ctr = 0
    for f in nc.m.functions:
        for bb in f.blocks:
            insts = bb.instructions
            if not any(
                i.sync_info is not None
                and i.sync_info.on_wait
                and len(i.sync_info.on_wait) > max_waits
                for i in insts
            ):
                continue
            new_list = []
            for inst in insts:
                si = inst.sync_info
                if si is not None and si.on_wait and len(si.on_wait) > max_waits:
                    waits = list(si.on_wait)
                    extra, keep = waits[:-max_waits], waits[-max_waits:]
                    for j in range(0, len(extra), max_waits):
                        ctr += 1
                        nop = mybir.InstNoOp(name=f"I-WSPLIT-{ctr}", ins=[], outs=[])
                        nop.engine = inst.engine
                        nop.sync_info = mybir.SyncInfo(
                            on_wait=extra[j : j + max_waits], on_update=[]
                        )
                        new_list.append(nop)
                    inst.sync_info = mybir.SyncInfo(
                        on_wait=keep, on_update=list(si.on_update or [])
                    )
                new_list.append(inst)
            bb.instructions[:] = new_list


@lru_cache(maxsize=2)
def _build(L_cache):
    import concourse.bass as bass
    import concourse.mybir as mybir
    import concourse.tile as tile
    from concourse.masks import make_identity

    f32 = mybir.dt.float32
    f32r = mybir.dt.float32r
    bf16 = mybir.dt.bfloat16
    AF = mybir.ActivationFunctionType
    ALU = mybir.AluOpType
    AX = mybir.AxisListType

    T = T_TOK
    L = L_cache + T
    SCALE = 1.0 / math.sqrt(HEAD_DIM)

    cache_ch = [(o, s) for (o, s) in _chunks(L_cache)]
    TCH = _chunks(T)
    NTC = len(TCH)
    LCH = cache_ch + [(L_cache + o, s) for (o, s) in TCH]
    NLC = len(LCH)
    NVC = len(cache_ch)
    T_PAD = T + (T % 2)  # fp32r matmuls need an even moving size
    HALVES = [(0, 512), (512, T - 512)]
    HMAX = max(s + s % 2 for _, s in HALVES)
    NST = 2 * NTC * 4  # AllReduce staging width (both batch slots)

    nc = bass.Bass(num_devices=N_CORES)

    xT = nc.declare_dram_parameter("xT", [N_EMBD, T], f32r, isOutput=False)
    wqT = nc.declare_dram_parameter("wqT", [N_EMBD, CPG], f32r, isOutput=False)
    wkT = nc.declare_dram_parameter("wkT", [N_EMBD, CPG], f32r, isOutput=False)
    wvT = nc.declare_dram_parameter("wvT", [N_EMBD, CPG], f32r, isOutput=False)
    wpT = nc.declare_dram_parameter("wpT", [CPG, N_EMBD], bf16, isOutput=False)
    kTc = nc.declare_dram_parameter(
        "kTc", [HPG, HEAD_DIM, max(L_cache, 1)], bf16, isOutput=False
    )
    vc = nc.declare_dram_parameter(
        "vc", [HPG, max(L_cache, 1), HEAD_DIM], bf16, isOutput=False
    )
    cosN = nc.declare_dram_parameter("cosN", [T, CPG], f32, isOutput=False)
    sinN = nc.declare_dram_parameter("sinN", [T, CPG], f32, isOutput=False)
    qn_w = nc.declare_dram_parameter("qn_w", [CPG], f32, isOutput=False)
    qn_b = nc.declare_dram_parameter("qn_b", [CPG], f32, isOutput=False)
    kn_w = nc.declare_dram_parameter("kn_w", [CPG], f32, isOutput=False)
    kn_b = nc.declare_dram_parameter("kn_b", [CPG], f32, isOutput=False)
    bmask = nc.declare_dram_parameter("bmask", [128, 2], f32, isOutput=False)
    outp = nc.declare_dram_parameter("outp", [T, N_EMBD], f32, isOutput=True)

    ar_in = nc.dram_tensor("ar_in", [128, NST], f32)
    ar_out = nc.dram_tensor("ar_out", [128, NST], f32, addr_space="Shared")

    with tile.TileContext(nc) as tc:
        with tc.tile_pool(name="persist", bufs=1) as p1:
            qT = p1.tile([128, HPG, T_PAD], bf16)
            if T_PAD > T:
                nc.vector.memset(qT[:, :, T:T_PAD], 0.0)
            kN = p1.tile([128, HPG, T], bf16)
            vN = p1.tile([128, HPG, NTC, HEAD_DIM], bf16)
            ones_bf = p1.tile([128, 128], bf16)
            nc.vector.memset(ones_bf, 1.0)
            ident = p1.tile([128, 128], f32)
            make_identity(nc, ident)

            # ============ phase A: projections + stats + LN + rope ==========
            with tc.tile_pool(name="ln", bufs=1) as pln:
                z_sb = pln.tile([128, 2, NTC, CPG], f32)
                stl = pln.tile([128, NTC, 4], f32)  # qsum, qsq, ksum, ksq
                stg = pln.tile([128, NTC, 4], f32)
                stage = pln.tile([128, NST], f32)
                garp = pln.tile([128, NST], f32)
                t36 = pln.tile([128, NTC * 4], f32)
                mu2 = pln.tile([128, 2, NTC], f32)
                r2 = pln.tile([128, 2, NTC], f32)
                m2 = pln.tile([128, 2, NTC], f32)
                sc1 = pln.tile([128, NTC], f32)
                sc2 = pln.tile([128, NTC], f32)
                sc3 = pln.tile([128, NTC], f32)
                epsT = pln.tile([128, 1], f32)
                nc.vector.memset(epsT, EPS)
                bmt = pln.tile([128, 2], f32)
                nc.sync.dma_start(out=bmt, in_=bmask[:, :])
                wbb = pln.tile([128, 2, 2, CPG], f32)  # (q/k, w/b) bcast
                for i, src in enumerate((qn_w, qn_b, kn_w, kn_b)):
                    sap = src[:]
                    nc.sync.dma_start(
                        out=wbb[:, i // 2, i % 2, :],
                        in_=bass.AP(
                            tensor=sap.tensor,
                            offset=sap.offset,
                            ap=[[0, 128]] + list(sap.ap),
                        ),
                    )
                nc.vector.memset(stl, 0.0)

                with tc.tile_pool(name="proj", bufs=1) as pa, \
                     tc.tile_pool(name="sqs", bufs=2) as psq, \
                     tc.tile_pool(name="ps_z", bufs=3, space="PSUM") as ps_z:
                    xs = pa.tile([128, NCC, T], f32r)
                    nc.sync.dma_start(
                        out=xs, in_=xT.rearrange("(c p) t -> p c t", p=128)
                    )

                    # --- q, k projections + per-token stats ---
                    for pi, wsrc in enumerate((wqT, wkT)):
                        w_t = pa.tile([128, NCC, CPG], f32r, tag="w")
                        nc.sync.dma_start(
                            out=w_t, in_=wsrc.rearrange("(c p) n -> p c n", p=128)
                        )
                        for tci, (t0, tsz) in enumerate(TCH):
                            zp = ps_z.tile([128, CPG], f32, tag="zp")
                            for cc in range(NCC):
                                nc.tensor.matmul(
                                    zp[:tsz, :],
                                    lhsT=xs[:, cc, t0 : t0 + tsz],
                                    rhs=w_t[:, cc, :],
                                    start=(cc == 0),
                                    stop=(cc == NCC - 1),
                                )
                            dst = z_sb[:tsz, pi, tci, :]
                            nc.vector.tensor_copy(dst, zp[:tsz, :])
                            sq = psq.tile([128, CPG], f32, tag="sq")
                            nc.scalar.activation(
                                sq[:tsz, :],
                                zp[:tsz, :],
                                AF.Square,
                                accum_out=stl[:tsz, tci, 2 * pi + 1 : 2 * pi + 2],
                            )
                            nc.vector.tensor_reduce(
                                stl[:tsz, tci, 2 * pi : 2 * pi + 1],
                                dst,
                                axis=AX.X,
                                op=ALU.add,
                            )

                    # --- masked 8-core AllReduce of the stats ---
                    stl_f = stl.rearrange("p a b -> p (a b)")
                    nc.vector.tensor_scalar(
                        stage[:, 0 : NST // 2], stl_f, bmt[:, 0:1], None, op0=ALU.mult
                    )
                    nc.vector.tensor_scalar(
                        stage[:, NST // 2 : NST], stl_f, bmt[:, 1:2], None,
                        op0=ALU.mult,
                    )
                    nc.sync.dma_start(out=ar_in[:, :], in_=stage)
                    nc.gpsimd.collective_compute(
                        "AllReduce",
                        ALU.add,
                        replica_groups=[[0, 1, 2, 3, 4, 5, 6, 7]],
                        ins=[ar_in[:, :]],
                        outs=[ar_out[:, :]],
                    )
                    garp_ = garp
                    nc.sync.dma_start(out=garp_, in_=ar_out[:, :])
                    stg_f = stg.rearrange("p a b -> p (a b)")
                    nc.vector.tensor_scalar(
                        t36, garp_[:, 0 : NST // 2], bmt[:, 0:1], None, op0=ALU.mult
                    )
                    nc.vector.tensor_scalar(
                        stg_f, garp_[:, NST // 2 : NST], bmt[:, 1:2], None,
                        op0=ALU.mult,
                    )
                    nc.vector.tensor_add(stg_f, stg_f, t36)

                    # --- v projection (overlaps the collective) ---
                    with tc.tile_pool(name="ps_v", bufs=2, space="PSUM") as ps_v:
                        wv_t = pa.tile([128, NCC, CPG], f32r, tag="w")
                        nc.sync.dma_start(
                            out=wv_t, in_=wvT.rearrange("(c p) n -> p c n", p=128)
                        )
                        for tci, (t0, tsz) in enumerate(TCH):
                            vp = ps_v.tile([128, CPG], f32)
                            for cc in range(NCC):
                                nc.tensor.matmul(
                                    vp[:tsz, :],
                                    lhsT=xs[:, cc, t0 : t0 + tsz],
                                    rhs=wv_t[:, cc, :],
                                    start=(cc == 0),
                                    stop=(cc == NCC - 1),
                                )
                            nc.vector.tensor_copy(
                                vN[:tsz, :, tci, :],
                                vp[:tsz, :].rearrange("p (h d) -> p h d", h=HPG),
                            )

                    # --- LN scalar math: mu, rstd, mu*rstd  [128, NTC] each ---
                    for pi in range(2):
                        nc.vector.tensor_scalar(
                            mu2[:, pi, :], stg[:, :, 2 * pi], 1.0 / N_EMBD, None,
                            op0=ALU.mult,
                        )
                        nc.vector.tensor_scalar(
                            sc1, stg[:, :, 2 * pi + 1], 1.0 / N_EMBD, None,
                            op0=ALU.mult,
                        )
                        nc.vector.tensor_mul(sc2, mu2[:, pi, :], mu2[:, pi, :])
                        nc.vector.tensor_sub(sc3, sc1, sc2)
                        nc.scalar.activation(sc1, sc3, AF.Sqrt, bias=epsT)
                        nc.vector.reciprocal(r2[:, pi, :], sc1)
                        nc.vector.tensor_mul(m2[:, pi, :], mu2[:, pi, :], r2[:, pi, :])

                # --- LN apply + rope + transpose to (d, t) layout ---
                with tc.tile_pool(name="rope", bufs=2) as pr, \
                     tc.tile_pool(name="rtmp", bufs=2) as prt, \
                     tc.tile_pool(name="ps_t", bufs=4, space="PSUM") as ps_t:
                    for tci, (t0, tsz) in enumerate(TCH):
                        cs = pr.tile([128, CPG], f32, tag="cos")
                        nc.sync.dma_start(out=cs[:tsz, :], in_=cosN[t0 : t0 + tsz, :])
                        sn = pr.tile([128, CPG], f32, tag="sin")
                        nc.sync.dma_start(out=sn[:tsz, :], in_=sinN[t0 : t0 + tsz, :])
                        for pi in range(2):
                            z = z_sb[:tsz, pi, tci, :]
                            t1 = prt.tile([128, CPG], f32, tag="A")
                            nc.vector.tensor_scalar(
                                t1[:tsz, :],
                                z,
                                mu2[:tsz, pi, tci : tci + 1],
                                r2[:tsz, pi, tci : tci + 1],
                                op0=ALU.subtract,
                                op1=ALU.mult,
                            )
                            t2 = prt.tile([128, CPG], f32, tag="B")
                            nc.vector.tensor_mul(
                                t2[:tsz, :], t1[:tsz, :], wbb[:tsz, pi, 0, :]
                            )
                            t3 = prt.tile([128, CPG], f32, tag="A")
                            nc.vector.tensor_add(
                                t3[:tsz, :], t2[:tsz, :], wbb[:tsz, pi, 1, :]
                            )
                            sw = prt.tile([128, CPG], f32, tag="B")
                            swr = sw.rearrange("p (i two) -> p i two", two=2)
                            t3r = t3.rearrange("p (i two) -> p i two", two=2)
                            nc.vector.tensor_copy(
                                swr[:tsz, :, 0:1], t3r[:tsz, :, 1:2]
                            )
                            nc.vector.tensor_copy(
                                swr[:tsz, :, 1:2], t3r[:tsz, :, 0:1]
                            )
                            a = prt.tile([128, CPG], f32, tag="A2")
                            nc.vector.tensor_mul(a[:tsz, :], t3[:tsz, :], cs[:tsz, :])
                            b = prt.tile([128, CPG], f32, tag="B2")
                            nc.vector.tensor_mul(b[:tsz, :], sw[:tsz, :], sn[:tsz, :])
                            r = prt.tile([128, CPG], f32, tag="C")
                            nc.vector.tensor_add(r[:tsz, :], a[:tsz, :], b[:tsz, :])
                            dstq = (qT if pi == 0 else kN)
                            for ci in range(HPG):
                                tp = ps_t.tile([128, 128], f32)
                                nc.tensor.transpose(
                                    tp[:, :tsz],
                                    r[:tsz, ci * 128 : (ci + 1) * 128],
                                    ident[:tsz, :tsz],
                                )
                                nc.vector.tensor_copy(
                                    dstq[:, ci, t0 : t0 + tsz], tp[:, :tsz]
                                )

            # ================= phase B: attention ===========================
            with tc.tile_pool(name="yt", bufs=1) as py:
                yT = py.tile([128, HPG, T], bf16)
                wp_sb = py.tile([128, HPG, N_EMBD], bf16)
                nc.sync.dma_start(
                    out=wp_sb, in_=wpT.rearrange("(h p) n -> p h n", p=128)
                )
                with tc.tile_pool(name="kt", bufs=2) as pb_kt, \
                     tc.tile_pool(name="vv", bufs=2) as pb_v, \
                     tc.tile_pool(name="pt", bufs=2) as pb_pt, \
                     tc.tile_pool(name="rcp", bufs=2) as pb_rc, \
                     tc.tile_pool(name="ps_s", bufs=2, space="PSUM") as ps_s, \
                     tc.tile_pool(name="ps_av", bufs=1, space="PSUM") as ps_av, \
                     tc.tile_pool(name="ps_dn", bufs=1, space="PSUM") as ps_dn:
                    for h in range(HPG):
                        KT_h = pb_kt.tile([128, L], bf16)
                        if L_cache > 0:
                            nc.sync.dma_start(out=KT_h[:, :L_cache], in_=kTc[h])
                        nc.vector.tensor_copy(KT_h[:, L_cache:], kN[:, h, :])
                        V_h = pb_v.tile([128, NLC, HEAD_DIM], bf16)
                        ncf, rem = L_cache // 128, L_cache % 128
                        if ncf:
                            nc.sync.dma_start(
                                out=V_h[:, 0:ncf, :],
                                in_=vc[h, 0 : ncf * 128, :].rearrange(
                                    "(c p) d -> p c d", p=128
                                ),
                            )
                        if rem:
                            nc.sync.dma_start(
                                out=V_h[0:rem, ncf, :],
                                in_=vc[h, ncf * 128 : L_cache, :],
                            )
                        for tci, (t0, tsz) in enumerate(TCH):
                            nc.vector.tensor_copy(
                                V_h[:tsz, NVC + tci, :], vN[:tsz, h, tci, :]
                            )
                        for (t0, tsz) in HALVES:
                            PT = pb_pt.tile([128, NLC, HMAX], bf16)
                            for li, (l0, lsz) in enumerate(LCH):
                                sp = ps_s.tile([128, HMAX], f32)
                                for (m0, msz) in _chunks(tsz + tsz % 2, 512):
                                    nc.tensor.matmul(
                                        sp[0:lsz, m0 : m0 + msz],
                                        lhsT=KT_h[:, l0 : l0 + lsz],
                                        rhs=qT[
                                            :, h, t0 + m0 : t0 + m0 + msz
                                        ],
                                        start=True,
                                        stop=True,
                                    )
                                nc.scalar.activation(
                                    PT[0:lsz, li, 0:tsz],
                                    sp[0:lsz, 0:tsz],
                                    AF.Exp,
                                    scale=SCALE,
                                )
                            av = ps_av.tile([128, HMAX], f32)
                            den = ps_dn.tile([128, HMAX], f32)
                            for li, (l0, lsz) in enumerate(LCH):
                                st, sp_ = (li == 0), (li == NLC - 1)
                                for (m0, msz) in _chunks(tsz, 512):
                                    nc.tensor.matmul(
                                        av[:, m0 : m0 + msz],
                                        lhsT=V_h[0:lsz, li, :],
                                        rhs=PT[0:lsz, li, m0 : m0 + msz],
                                        start=st,
                                        stop=sp_,
                                    )
                            for li, (l0, lsz) in enumerate(LCH):
                                st, sp_ = (li == 0), (li == NLC - 1)
                                for (m0, msz) in _chunks(tsz, 512):
                                    nc.tensor.matmul(
                                        den[:, m0 : m0 + msz],
                                        lhsT=ones_bf[0:lsz, :],
                                        rhs=PT[0:lsz, li, m0 : m0 + msz],
                                        start=st,
                                        stop=sp_,
                                    )
                            rc = pb_rc.tile([128, HMAX], f32)
                            nc.vector.reciprocal(rc[:, 0:tsz], den[:, 0:tsz])
                            nc.vector.tensor_mul(
                                yT[:, h, t0 : t0 + tsz], av[:, 0:tsz], rc[:, 0:tsz]
                            )

                # ================= phase C: out-projection ==================
                with tc.tile_pool(name="oc", bufs=2) as poc, \
                     tc.tile_pool(name="ps_o", bufs=2, space="PSUM") as ps_o:
                    for (t0, tsz) in TCH:
                        op = ps_o.tile([128, N_EMBD], f32)
                        for co in range(4):
                            for h in range(HPG):
                                nc.tensor.matmul(
                                    op[0:tsz, co * 512 : (co + 1) * 512],
                                    lhsT=yT[:, h, t0 : t0 + tsz],
                                    rhs=wp_sb[:, h, co * 512 : (co + 1) * 512],
                                    start=(h == 0),
                                    stop=(h == HPG - 1),
                                )
                        ot = poc.tile([128, N_EMBD], f32)
                        nc.vector.tensor_copy(ot[0:tsz, :], op[0:tsz, :])
                        nc.sync.dma_start(out=outp[t0 : t0 + tsz, :], in_=ot[0:tsz, :])

    import concourse.mybir as mybir_mod

    _split_excess_waits(nc, mybir_mod, max_waits=1)
    return nc


def _window_rows(T, cache_size, current_start, global_end_index, local_end_index):
    """Replicates the reference's sink+sliding-window eviction arithmetic;
    returns the cache row indices of the attention window (new rows follow)."""
    current_end = current_start + T
    if current_end > global_end_index and T + local_end_index > cache_size:
        num_evicted = T + local_end_index - cache_size
        num_rolled = local_end_index - num_evicted - SINK_TOKENS
        new_local_end = (
            local_end_index + current_end - global_end_index - num_evicted
        )
        cache_rows = list(range(SINK_TOKENS)) + list(
            range(SINK_TOKENS + num_evicted, SINK_TOKENS + num_evicted + num_rolled)
        )
    else:
        new_local_end = local_end_index + current_end - global_end_index
        cache_rows = list(range(new_local_end - T))
    cache_start = max(0, new_local_end - MAX_ATTN)
    m = len(cache_rows)
    assert cache_start <= m and new_local_end - m == T, (
        "kernel supports windows that contain all new tokens"
    )
    return cache_rows[cache_start:m]


def kernel(**inputs):
    global LAST_EXEC_NS
    from concourse.bass_utils import run_bass_kernel_spmd

    x = np.asarray(inputs["x"], np.float32)
    Wq = np.asarray(inputs["Wq"], np.float32)
    Wk = np.asarray(inputs["Wk"], np.float32)
    Wv = np.asarray(inputs["Wv"], np.float32)
    Wproj = np.asarray(inputs["Wproj"], np.float32)
    qn_w = np.asarray(inputs["qn_w"], np.float32)
    qn_b = np.asarray(inputs["qn_b"], np.float32)
    kn_w = np.asarray(inputs["kn_w"], np.float32)
    kn_b = np.asarray(inputs["kn_b"], np.float32)
    cache_k = np.asarray(inputs["cache_k"], np.float32)
    cache_v = np.asarray(inputs["cache_v"], np.float32)
    cs = int(inputs["current_start"])
    ge = int(inputs["global_end_index"])
    le = int(inputs["local_end_index"])

    Bsz, T, C = x.shape
    assert (Bsz, T, C) == (2, T_TOK, N_EMBD)
    win = np.asarray(
        _window_rows(T, cache_k.shape[1], cs, ge, le), dtype=np.int64
    )
    L_cache = len(win)

    nc = _build(L_cache)

    cosN, sinN = _rope_tables()
    in_maps = []
    for core in range(N_CORES):
        b, hg = core // 4, core % 4
        ch0, hs = hg * CPG, hg * HPG
        kc = (
            cache_k[b][win][:, hs : hs + HPG, :]
            if L_cache
            else np.zeros((1, HPG, HEAD_DIM), np.float32)
        )
        vv = (
            cache_v[b][win][:, hs : hs + HPG, :]
            if L_cache
            else np.zeros((1, HPG, HEAD_DIM), np.float32)
        )
        bm = np.zeros((128, 2), np.float32)
        bm[:, b] = 1.0
        in_maps.append(
            {
                "xT": np.ascontiguousarray(x[b].T),
                "wqT": np.ascontiguousarray(Wq[ch0 : ch0 + CPG, :].T),
                "wkT": np.ascontiguousarray(Wk[ch0 : ch0 + CPG, :].T),
                "wvT": np.ascontiguousarray(Wv[ch0 : ch0 + CPG, :].T),
                "wpT": np.ascontiguousarray(Wproj[:, ch0 : ch0 + CPG].T).astype(
                    ml_dtypes.bfloat16
                ),
                "kTc": np.ascontiguousarray(kc.transpose(1, 2, 0)).astype(
                    ml_dtypes.bfloat16
                ),
                "vc": np.ascontiguousarray(vv.transpose(1, 0, 2)).astype(
                    ml_dtypes.bfloat16
                ),
                "cosN": cosN,
                "sinN": sinN,
                "qn_w": np.ascontiguousarray(qn_w[ch0 : ch0 + CPG]),
                "qn_b": np.ascontiguousarray(qn_b[ch0 : ch0 + CPG]),
                "kn_w": np.ascontiguousarray(kn_w[ch0 : ch0 + CPG]),
                "kn_b": np.ascontiguousarray(kn_b[ch0 : ch0 + CPG]),
                "bmask": bm,
            }
        )

    trace = os.environ.get("TRN_KERNEL_TRACE", "0") == "1"
    res = run_bass_kernel_spmd(
        nc, in_maps, core_ids=list(range(N_CORES)), trace=trace
    )
    if trace:
        LAST_EXEC_NS = res.exec_time_ns

    out = np.zeros((Bsz, T, C), np.float32)
    for core in range(N_CORES):
        out[core // 4] += res.results[core]["outp"]
    return out


# revision 18
# speedup vs baseline: 1.1793x; 1.1731x over previous
"""Trainium2 Bass kernel for CausalSelfAttentionWithCache (sink+window KV cache).

Sharding: 8 cores = batch(2) x head-group(4). Each core owns one batch element
and 4 heads (512 channels): Megatron column-parallel Wq/Wk/Wv, row-parallel
Wproj (the 4 head-group partial outputs are summed on the host). QK-LayerNorm
is over the full 2048 channels, so per-token sum / sum-of-squares are
AllReduced on-device across the 4 cores of each batch group (8-core AllReduce,
batch selected by a per-core mask input; <=4-core shared-output collectives
are unsupported, and compute engines can only address partition bases
{0,32,64}, so batch slots live in the free dimension).

Device pipeline per core:
  A) q/k/v projections in natural [token, channel] layout (x^T chunks
     stationary, W^T moving, fp32r), per-token LN stats via free-dim reduces
     (ScalarE Square+accum / VectorE reduce), tiny [128,72] AllReduce, LN
     apply + partial 2D-axial RoPE on VectorE, then PE transposes into the
     q^T / k^T (d, t) layout that attention needs.
  B) attention per (head, t-half): S^T = K^T-chunk-stationary @ q^T-moving
     (fp32r), exp on ScalarE with the 1/sqrt(128) scale fused (no max
     subtraction -- logits are O(1) by construction), P^T bf16 resident in
     SBUF; AV and the softmax denominator as V-/ones-stationary matmuls over
     the same P^T moving stream, PSUM-accumulated over l-chunks; normalized
     with VectorE reciprocal. The attention output O^T doubles as y^T.
  C) out-projection: y^T chunks stationary, Wproj^T moving, fp32r; partial
     [1035, 2048] DMA'd out per core.
"""

import math
import os
from functools import lru_cache

import ml_dtypes
import numpy as np

# ---- problem constants ----
N_HEAD = 16
HEAD_DIM = 128
N_EMBD = 2048
PREFIX = 11
PATCH_X, PATCH_Y = 32, 32
T_TOK = PREFIX + PATCH_X * PATCH_Y  # 1035
SINK_TOKENS = T_TOK
MAX_ATTN = 4 * T_TOK  # 4140
THETA = 1000.0
EPS = 1e-5

N_CORES = 8
HPG = 4                      # heads per group (core)
CPG = HPG * HEAD_DIM         # 512 channels per core
NCC = N_EMBD // 128          # 16 contraction chunks

LAST_EXEC_NS = None  # set when TRN_KERNEL_TRACE=1


def _chunks(total, step=128):
    out = []
    o = 0
    while o < total:
        out.append((o, min(step, total - o)))
        o += step
    return out


def _rope_tables():
    """cosN/sinN [T_TOK, CPG] fp32, natural (token, channel) layout.

    Column ch: pair index i = (ch % 128)//2 (same for every head). t < PREFIX:
    identity (cos=1, sin=0). sinN sign baked in: even cols -sin, odd cols +sin,
    pairing with an adjacent-channel swap to realize the 2x2 rotation.
    """
    dim = HEAD_DIM
    n = dim // 4
    freqs = 1.0 / (THETA ** (np.arange(0, dim, 4)[:n].astype(np.float32) / dim))
    t = np.arange(PATCH_X * PATCH_Y, dtype=np.float32)
    tx = t % PATCH_X
    ty = np.floor(t / PATCH_X)
    ang = np.concatenate([np.outer(tx, freqs), np.outer(ty, freqs)], axis=-1)
    cos = np.cos(ang)  # [1024, 64]
    sin = np.sin(ang)
    cosN = np.ones((T_TOK, CPG), np.float32)
    sinN = np.zeros((T_TOK, CPG), np.float32)
    ch = np.arange(CPG)
    pair = (ch % HEAD_DIM) // 2
    cosN[PREFIX:, :] = cos[:, pair]
    sign = np.where(ch % 2 == 0, -1.0, 1.0).astype(np.float32)[None, :]
    sinN[PREFIX:, :] = sin[:, pair] * sign
    return cosN, sinN


def _split_excess_waits(nc, mybir, max_waits=1):
    """This walrus build caps semaphore waits per CTRL instruction. Move the
    excess onto preceding same-engine NoOps (semantically identical)."""
    ctr = 0
    for f in nc.m.functions:
        for bb in f.blocks:
            insts = bb.instructions
            if not any(
                i.sync_info is not None
                and i.sync_info.on_wait
                and len(i.sync_info.on_wait) > max_waits
                for i in insts
            ):
                continue
            new_list = []
            for inst in insts:
                si = inst.sync_info
                if si is not None and si.on_wait and len(si.on_wait) > max_waits:
                    waits = list(si.on_wait)
                    extra, keep = waits[:-max_waits], waits[-max_waits:]
                    for j in range(0, len(extra), max_waits):
                        ctr += 1
                        nop = mybir.InstNoOp(name=f"I-WSPLIT-{ctr}", ins=[], outs=[])
                        nop.engine = inst.engine
                        nop.sync_info = mybir.SyncInfo(
                            on_wait=extra[j : j + max_waits], on_update=[]
                        )
                        new_list.append(nop)
                    inst.sync_info = mybir.SyncInfo(
                        on_wait=keep, on_update=list(si.on_update or [])
                    )
                new_list.append(inst)
            bb.instructions[:] = new_list


@lru_cache(maxsize=2)
def _build(L_cache):
    import concourse.bass as bass
    import concourse.mybir as mybir
    import concourse.tile as tile
    from concourse.masks import make_identity

    f32 = mybir.dt.float32
    f32r = mybir.dt.float32r
    bf16 = mybir.dt.bfloat16
    AF = mybir.ActivationFunctionType
    ALU = mybir.AluOpType
    AX = mybir.AxisListType

    T = T_TOK
    L = L_cache + T
    SCALE = 1.0 / math.sqrt(HEAD_DIM)

    cache_ch = [(o, s) for (o, s) in _chunks(L_cache)]
    TCH = _chunks(T)
    NTC = len(TCH)
    LCH = cache_ch + [(L_cache + o, s) for (o, s) in TCH]
    NLC = len(LCH)
    NVC = len(cache_ch)
    T_PAD = T + (T % 2)  # fp32r matmuls need an even moving size
    HALVES = [(0, 512), (512, T - 512)]
    HMAX = max(s + s % 2 for _, s in HALVES)
    NST = 2 * NTC * 4  # AllReduce staging width (both batch slots)

    nc = bass.Bass(num_devices=N_CORES)

    xT = nc.declare_dram_parameter("xT", [N_EMBD, T], bf16, isOutput=False)
    wqT = nc.declare_dram_parameter("wqT", [N_EMBD, CPG], bf16, isOutput=False)
    wkT = nc.declare_dram_parameter("wkT", [N_EMBD, CPG], bf16, isOutput=False)
    wvT = nc.declare_dram_parameter("wvT", [N_EMBD, CPG], bf16, isOutput=False)
    wpT = nc.declare_dram_parameter("wpT", [CPG, N_EMBD], bf16, isOutput=False)
    kTc = nc.declare_dram_parameter(
        "kTc", [HPG, HEAD_DIM, max(L_cache, 1)], bf16, isOutput=False
    )
    vc = nc.declare_dram_parameter(
        "vc", [HPG, max(L_cache, 1), HEAD_DIM], bf16, isOutput=False
    )
    cosN = nc.declare_dram_parameter("cosN", [T, CPG], bf16, isOutput=False)
    sinN = nc.declare_dram_parameter("sinN", [T, CPG], bf16, isOutput=False)
    qn_w = nc.declare_dram_parameter("qn_w", [CPG], bf16, isOutput=False)
    qn_b = nc.declare_dram_parameter("qn_b", [CPG], bf16, isOutput=False)
    kn_w = nc.declare_dram_parameter("kn_w", [CPG], bf16, isOutput=False)
    kn_b = nc.declare_dram_parameter("kn_b", [CPG], bf16, isOutput=False)
    bmask = nc.declare_dram_parameter("bmask", [128, 2], f32, isOutput=False)
    outp = nc.declare_dram_parameter("outp", [T, N_EMBD], f32, isOutput=True)

    ar_in = nc.dram_tensor("ar_in", [128, NST], f32)
    ar_out = nc.dram_tensor("ar_out", [128, NST], f32, addr_space="Shared")

    with tile.TileContext(nc) as tc:
        with tc.tile_pool(name="persist", bufs=1) as p1:
            qT = p1.tile([128, HPG, T_PAD], bf16)
            if T_PAD > T:
                nc.vector.memset(qT[:, :, T:T_PAD], 0.0)
            kN = p1.tile([128, HPG, T], bf16)
            vN = p1.tile([128, HPG, NTC, HEAD_DIM], bf16)
            ones_bf = p1.tile([128, 128], bf16)
            nc.vector.memset(ones_bf, 1.0)
            ident = p1.tile([128, 128], bf16)
            make_identity(nc, ident)

            # ============ phase A: projections + stats + LN + rope ==========
            with tc.tile_pool(name="ln", bufs=1) as pln:
                z_sb = pln.tile([128, 2, NTC, CPG], bf16)
                stl = pln.tile([128, NTC, 4], f32)  # qsum, qsq, ksum, ksq
                stg = pln.tile([128, NTC, 4], f32)
                stage = pln.tile([128, NST], f32)
                garp = pln.tile([128, NST], f32)
                t36 = pln.tile([128, NTC * 4], f32)
                mu2 = pln.tile([128, 2, NTC], f32)
                r2 = pln.tile([128, 2, NTC], f32)
                m2 = pln.tile([128, 2, NTC], f32)
                m2n = pln.tile([128, 2, NTC], f32)
                sc1 = pln.tile([128, NTC], f32)
                sc2 = pln.tile([128, NTC], f32)
                sc3 = pln.tile([128, NTC], f32)
                epsT = pln.tile([128, 1], f32)
                nc.vector.memset(epsT, EPS)
                bmt = pln.tile([128, 2], f32)
                nc.sync.dma_start(out=bmt, in_=bmask[:, :])
                wbb = pln.tile([128, 2, 2, CPG], bf16)  # (q/k, w/b) bcast
                for i, src in enumerate((qn_w, qn_b, kn_w, kn_b)):
                    sap = src[:]
                    nc.sync.dma_start(
                        out=wbb[:, i // 2, i % 2, :],
                        in_=bass.AP(
                            tensor=sap.tensor,
                            offset=sap.offset,
                            ap=[[0, 128]] + list(sap.ap),
                        ),
                    )
                nc.vector.memset(stl, 0.0)

                with tc.tile_pool(name="proj", bufs=1) as pa, \
                     tc.tile_pool(name="pw", bufs=24) as pw, \
                     tc.tile_pool(name="sqs", bufs=2) as psq, \
                     tc.tile_pool(name="ps_z", bufs=3, space="PSUM") as ps_z:
                    xT_r = xT.rearrange("(c p) t -> p c t", p=128)
                    xs_t = []
                    for cc in range(NCC):
                        xc = pa.tile([128, T], bf16, tag=f"xs{cc}")
                        nc.sync.dma_start(out=xc, in_=xT_r[:, cc, :])
                        xs_t.append(xc)

                    # --- q, k projections + per-token stats ---
                    for pi, wsrc in enumerate((wqT, wkT)):
                        w_r = wsrc.rearrange("(c p) n -> p c n", p=128)
                        w_t = []
                        for cc in range(NCC):
                            wc = pw.tile([128, CPG], bf16, tag="w")
                            nc.sync.dma_start(out=wc, in_=w_r[:, cc, :])
                            w_t.append(wc)
                        for tci, (t0, tsz) in enumerate(TCH):
                            zp = ps_z.tile([128, CPG], f32, tag="zp")
                            for cc in range(NCC):
                                nc.tensor.matmul(
                                    zp[:tsz, :],
                                    lhsT=xs_t[cc][:, t0 : t0 + tsz],
                                    rhs=w_t[cc],
                                    start=(cc == 0),
                                    stop=(cc == NCC - 1),
                                )
                            dst = z_sb[:tsz, pi, tci, :]
                            nc.vector.tensor_copy(dst, zp[:tsz, :])
                            sq = psq.tile([128, CPG], f32, tag="sq")
                            nc.scalar.activation(
                                sq[:tsz, :],
                                zp[:tsz, :],
                                AF.Square,
                                accum_out=stl[:tsz, tci, 2 * pi + 1 : 2 * pi + 2],
                            )
                            nc.vector.tensor_reduce(
                                stl[:tsz, tci, 2 * pi : 2 * pi + 1],
                                dst,
                                axis=AX.X,
                                op=ALU.add,
                            )

                    # --- masked 8-core AllReduce of the stats ---
                    stl_f = stl.rearrange("p a b -> p (a b)")
                    nc.vector.tensor_scalar(
                        stage[:, 0 : NST // 2], stl_f, bmt[:, 0:1], None, op0=ALU.mult
                    )
                    nc.vector.tensor_scalar(
                        stage[:, NST // 2 : NST], stl_f, bmt[:, 1:2], None,
                        op0=ALU.mult,
                    )
                    nc.sync.dma_start(out=ar_in[:, :], in_=stage)
                    nc.gpsimd.collective_compute(
                        "AllReduce",
                        ALU.add,
                        replica_groups=[[0, 1, 2, 3, 4, 5, 6, 7]],
                        ins=[ar_in[:, :]],
                        outs=[ar_out[:, :]],
                    )
                    garp_ = garp
                    nc.sync.dma_start(out=garp_, in_=ar_out[:, :])
                    stg_f = stg.rearrange("p a b -> p (a b)")
                    nc.vector.tensor_scalar(
                        t36, garp_[:, 0 : NST // 2], bmt[:, 0:1], None, op0=ALU.mult
                    )
                    nc.vector.tensor_scalar(
                        stg_f, garp_[:, NST // 2 : NST], bmt[:, 1:2], None,
                        op0=ALU.mult,
                    )
                    nc.vector.tensor_add(stg_f, stg_f, t36)

                    # --- v projection (overlaps the collective) ---
                    with tc.tile_pool(name="ps_v", bufs=2, space="PSUM") as ps_v:
                        wv_r = wvT.rearrange("(c p) n -> p c n", p=128)
                        wv_t = []
                        for cc in range(NCC):
                            wc = pw.tile([128, CPG], bf16, tag="w")
                            nc.sync.dma_start(out=wc, in_=wv_r[:, cc, :])
                            wv_t.append(wc)
                        for tci, (t0, tsz) in enumerate(TCH):
                            vp = ps_v.tile([128, CPG], f32)
                            for cc in range(NCC):
                                nc.tensor.matmul(
                                    vp[:tsz, :],
                                    lhsT=xs_t[cc][:, t0 : t0 + tsz],
                                    rhs=wv_t[cc],
                                    start=(cc == 0),
                                    stop=(cc == NCC - 1),
                                )
                            nc.vector.tensor_copy(
                                vN[:tsz, :, tci, :],
                                vp[:tsz, :].rearrange("p (h d) -> p h d", h=HPG),
                            )

                    # --- LN scalar math: mu, rstd, mu*rstd  [128, NTC] each ---
                    for pi in range(2):
                        nc.vector.tensor_scalar(
                            mu2[:, pi, :], stg[:, :, 2 * pi], 1.0 / N_EMBD, None,
                            op0=ALU.mult,
                        )
                        nc.vector.tensor_scalar(
                            sc1, stg[:, :, 2 * pi + 1], 1.0 / N_EMBD, None,
                            op0=ALU.mult,
                        )
                        nc.vector.tensor_mul(sc2, mu2[:, pi, :], mu2[:, pi, :])
                        nc.vector.tensor_sub(sc3, sc1, sc2)
                        nc.scalar.activation(sc1, sc3, AF.Sqrt, bias=epsT)
                        nc.vector.reciprocal(r2[:, pi, :], sc1)
                        nc.vector.tensor_mul(m2[:, pi, :], mu2[:, pi, :], r2[:, pi, :])
                        nc.vector.tensor_scalar(
                            m2n[:, pi, :], m2[:, pi, :], -1.0, None, op0=ALU.mult
                        )

                # --- LN apply + rope + transpose to (d, t) layout ---
                with tc.tile_pool(name="rope", bufs=2) as pr, \
                     tc.tile_pool(name="rtmp", bufs=2) as prt, \
                     tc.tile_pool(name="ps_t", bufs=4, space="PSUM") as ps_t:
                    for tci, (t0, tsz) in enumerate(TCH):
                        cs = pr.tile([128, CPG], bf16, tag="cos")
                        nc.sync.dma_start(out=cs[:tsz, :], in_=cosN[t0 : t0 + tsz, :])
                        sn = pr.tile([128, CPG], bf16, tag="sin")
                        nc.sync.dma_start(out=sn[:tsz, :], in_=sinN[t0 : t0 + tsz, :])
                        for pi in range(2):
                            z = z_sb[:tsz, pi, tci, :]
                            t1 = prt.tile([128, CPG], bf16, tag="A")
                            nc.scalar.activation(
                                t1[:tsz, :],
                                z,
                                AF.Identity,
                                bias=m2n[:tsz, pi, tci : tci + 1],
                                scale=r2[:tsz, pi, tci : tci + 1],
                            )
                            t2 = prt.tile([128, CPG], bf16, tag="B")
                            nc.vector.tensor_mul(
                                t2[:tsz, :], t1[:tsz, :], wbb[:tsz, pi, 0, :]
                            )
                            t3 = prt.tile([128, CPG], bf16, tag="A")
                            nc.vector.tensor_add(
                                t3[:tsz, :], t2[:tsz, :], wbb[:tsz, pi, 1, :]
                            )
                            sw = prt.tile([128, CPG], bf16, tag="B")
                            swr = sw.rearrange("p (i two) -> p i two", two=2)
                            t3r = t3.rearrange("p (i two) -> p i two", two=2)
                            nc.vector.tensor_copy(
                                swr[:tsz, :, 0:1], t3r[:tsz, :, 1:2]
                            )
                            nc.vector.tensor_copy(
                                swr[:tsz, :, 1:2], t3r[:tsz, :, 0:1]
                            )
                            a = prt.tile([128, CPG], bf16, tag="A2")
                            nc.vector.tensor_mul(a[:tsz, :], t3[:tsz, :], cs[:tsz, :])
                            b = prt.tile([128, CPG], bf16, tag="B2")
                            nc.vector.tensor_mul(b[:tsz, :], sw[:tsz, :], sn[:tsz, :])
                            r = prt.tile([128, CPG], bf16, tag="C")
                            nc.vector.tensor_add(r[:tsz, :], a[:tsz, :], b[:tsz, :])
                            dstq = (qT if pi == 0 else kN)
                            for ci in range(HPG):
                                tp = ps_t.tile([128, 128], bf16)
                                nc.tensor.transpose(
                                    tp[:, :tsz],
                                    r[:tsz, ci * 128 : (ci + 1) * 128],
                                    ident[:tsz, :tsz],
                                )
                                nc.vector.tensor_copy(
                                    dstq[:, ci, t0 : t0 + tsz], tp[:, :tsz]
                                )

            # ================= phase B: attention ===========================
            with tc.tile_pool(name="yt", bufs=1) as py:
                yT = py.tile([128, HPG, T], bf16)
                wp_sb = py.tile([128, HPG, N_EMBD], bf16)
                nc.sync.dma_start(
                    out=wp_sb, in_=wpT.rearrange("(h p) n -> p h n", p=128)
                )
                with tc.tile_pool(name="kt", bufs=2) as pb_kt, \
                     tc.tile_pool(name="vv", bufs=2) as pb_v, \
                     tc.tile_pool(name="pt", bufs=2) as pb_pt, \
                     tc.tile_pool(name="rcp", bufs=2) as pb_rc, \
                     tc.tile_pool(name="ps_s", bufs=2, space="PSUM") as ps_s, \
                     tc.tile_pool(name="ps_av", bufs=1, space="PSUM") as ps_av, \
                     tc.tile_pool(name="ps_dn", bufs=1, space="PSUM") as ps_dn:
                    for h in range(HPG):
                        KT_h = pb_kt.tile([128, L], bf16)
                        if L_cache > 0:
                            nc.sync.dma_start(out=KT_h[:, :L_cache], in_=kTc[h])
                        nc.vector.tensor_copy(KT_h[:, L_cache:], kN[:, h, :])
                        V_h = pb_v.tile([128, NLC, HEAD_DIM], bf16)
                        ncf, rem = L_cache // 128, L_cache % 128
                        if ncf:
                            nc.sync.dma_start(
                                out=V_h[:, 0:ncf, :],
                                in_=vc[h, 0 : ncf * 128, :].rearrange(
                                    "(c p) d -> p c d", p=128
                                ),
                            )
                        if rem:
                            nc.sync.dma_start(
                                out=V_h[0:rem, ncf, :],
                                in_=vc[h, ncf * 128 : L_cache, :],
                            )
                        for tci, (t0, tsz) in enumerate(TCH):
                            nc.vector.tensor_copy(
                                V_h[:tsz, NVC + tci, :], vN[:tsz, h, tci, :]
                            )
                        for (t0, tsz) in HALVES:
                            PT = pb_pt.tile([128, NLC, HMAX], bf16)
                            for li, (l0, lsz) in enumerate(LCH):
                                sp = ps_s.tile([128, HMAX], f32)
                                for (m0, msz) in _chunks(tsz + tsz % 2, 512):
                                    nc.tensor.matmul(
                                        sp[0:lsz, m0 : m0 + msz],
                                        lhsT=KT_h[:, l0 : l0 + lsz],
                                        rhs=qT[
                                            :, h, t0 + m0 : t0 + m0 + msz
                                        ],
                                        start=True,
                                        stop=True,
                                    )
                                nc.scalar.activation(
                                    PT[0:lsz, li, 0:tsz],
                                    sp[0:lsz, 0:tsz],
                                    AF.Exp,
                                    scale=SCALE,
                                )
                            av = ps_av.tile([128, HMAX], f32)
                            den = ps_dn.tile([128, HMAX], f32)
                            for li, (l0, lsz) in enumerate(LCH):
                                st, sp_ = (li == 0), (li == NLC - 1)
                                for (m0, msz) in _chunks(tsz, 512):
                                    nc.tensor.matmul(
                                        av[:, m0 : m0 + msz],
                                        lhsT=V_h[0:lsz, li, :],
                                        rhs=PT[0:lsz, li, m0 : m0 + msz],
                                        start=st,
                                        stop=sp_,
                                    )
                            for li, (l0, lsz) in enumerate(LCH):
                                st, sp_ = (li == 0), (li == NLC - 1)
                                for (m0, msz) in _chunks(tsz, 512):
                                    nc.tensor.matmul(
                                        den[:, m0 : m0 + msz],
                                        lhsT=ones_bf[0:lsz, :],
                                        rhs=PT[0:lsz, li, m0 : m0 + msz],
                                        start=st,
                                        stop=sp_,
                                    )
                            rc = pb_rc.tile([128, HMAX], f32)
                            nc.vector.reciprocal(rc[:, 0:tsz], den[:, 0:tsz])
                            nc.vector.tensor_mul(
                                yT[:, h, t0 : t0 + tsz], av[:, 0:tsz], rc[:, 0:tsz]
                            )

                # ================= phase C: out-projection ==================
                with tc.tile_pool(name="oc", bufs=2) as poc, \
                     tc.tile_pool(name="ps_o", bufs=2, space="PSUM") as ps_o:
                    for (t0, tsz) in TCH:
                        op = ps_o.tile([128, N_EMBD], f32)
                        for co in range(4):
                            for h in range(HPG):
                                nc.tensor.matmul(
                                    op[0:tsz, co * 512 : (co + 1) * 512],
                                    lhsT=yT[:, h, t0 : t0 + tsz],
                                    rhs=wp_sb[:, h, co * 512 : (co + 1) * 512],
                                    start=(h == 0),
                                    stop=(h == HPG - 1),
                                )
                        ot = poc.tile([128, N_EMBD], f32)
                        nc.vector.tensor_copy(ot[0:tsz, :], op[0:tsz, :])
                        nc.sync.dma_start(out=outp[t0 : t0 + tsz, :], in_=ot[0:tsz, :])

    import concourse.mybir as mybir_mod

    _split_excess_waits(nc, mybir_mod, max_waits=1)
    return nc


def _window_rows(T, cache_size, current_start, global_end_index, local_end_index):
    """Replicates the reference's sink+sliding-window eviction arithmetic;
    returns the cache row indices of the attention window (new rows follow)."""
    current_end = current_start + T
    if current_end > global_end_index and T + local_end_index > cache_size:
        num_evicted = T + local_end_index - cache_size
        num_rolled = local_end_index - num_evicted - SINK_TOKENS
        new_local_end = (
            local_end_index + current_end - global_end_index - num_evicted
        )
        cache_rows = list(range(SINK_TOKENS)) + list(
            range(SINK_TOKENS + num_evicted, SINK_TOKENS + num_evicted + num_rolled)
        )
    else:
        new_local_end = local_end_index + current_end - global_end_index
        cache_rows = list(range(new_local_end - T))
    cache_start = max(0, new_local_end - MAX_ATTN)
    m = len(cache_rows)
    assert cache_start <= m and new_local_end - m == T, (
        "kernel supports windows that contain all new tokens"
    )
    return cache_rows[cache_start:m]


def kernel(**inputs):
    global LAST_EXEC_NS
    from concourse.bass_utils import run_bass_kernel_spmd

    x = np.asarray(inputs["x"], np.float32)
    Wq = np.asarray(inputs["Wq"], np.float32)
    Wk = np.asarray(inputs["Wk"], np.float32)
    Wv = np.asarray(inputs["Wv"], np.float32)
    Wproj = np.asarray(inputs["Wproj"], np.float32)
    qn_w = np.asarray(inputs["qn_w"], np.float32)
    qn_b = np.asarray(inputs["qn_b"], np.float32)
    kn_w = np.asarray(inputs["kn_w"], np.float32)
    kn_b = np.asarray(inputs["kn_b"], np.float32)
    cache_k = np.asarray(inputs["cache_k"], np.float32)
    cache_v = np.asarray(inputs["cache_v"], np.float32)
    cs = int(inputs["current_start"])
    ge = int(inputs["global_end_index"])
    le = int(inputs["local_end_index"])

    Bsz, T, C = x.shape
    assert (Bsz, T, C) == (2, T_TOK, N_EMBD)
    win = np.asarray(
        _window_rows(T, cache_k.shape[1], cs, ge, le), dtype=np.int64
    )
    L_cache = len(win)

    nc = _build(L_cache)

    cosN, sinN = _rope_tables()
    in_maps = []
    for core in range(N_CORES):
        b, hg = core // 4, core % 4
        ch0, hs = hg * CPG, hg * HPG
        kc = (
            cache_k[b][win][:, hs : hs + HPG, :]
            if L_cache
            else np.zeros((1, HPG, HEAD_DIM), np.float32)
        )
        vv = (
            cache_v[b][win][:, hs : hs + HPG, :]
            if L_cache
            else np.zeros((1, HPG, HEAD_DIM), np.float32)
        )
        bm = np.zeros((128, 2), np.float32)
        bm[:, b] = 1.0
        in_maps.append(
            {
                "xT": np.ascontiguousarray(x[b].T).astype(ml_dtypes.bfloat16),
                "wqT": np.ascontiguousarray(Wq[ch0 : ch0 + CPG, :].T).astype(
                    ml_dtypes.bfloat16
                ),
                "wkT": np.ascontiguousarray(Wk[ch0 : ch0 + CPG, :].T).astype(
                    ml_dtypes.bfloat16
                ),
                "wvT": np.ascontiguousarray(Wv[ch0 : ch0 + CPG, :].T).astype(
                    ml_dtypes.bfloat16
                ),
                "wpT": np.ascontiguousarray(Wproj[:, ch0 : ch0 + CPG].T).astype(
                    ml_dtypes.bfloat16
                ),
                "kTc": np.ascontiguousarray(kc.transpose(1, 2, 0)).astype(
                    ml_dtypes.bfloat16
                ),
                "vc": np.ascontiguousarray(vv.transpose(1, 0, 2)).astype(
                    ml_dtypes.bfloat16
                ),
                "cosN": cosN.astype(ml_dtypes.bfloat16),
                "sinN": sinN.astype(ml_dtypes.bfloat16),
                "qn_w": np.ascontiguousarray(qn_w[ch0 : ch0 + CPG]).astype(
                    ml_dtypes.bfloat16
                ),
                "qn_b": np.ascontiguousarray(qn_b[ch0 : ch0 + CPG]).astype(
                    ml_dtypes.bfloat16
                ),
                "kn_w": np.ascontiguousarray(kn_w[ch0 : ch0 + CPG]).astype(
                    ml_dtypes.bfloat16
                ),
                "kn_b": np.ascontiguousarray(kn_b[ch0 : ch0 + CPG]).astype(
                    ml_dtypes.bfloat16
                ),
                "bmask": bm,
            }
        )

    trace = os.environ.get("TRN_KERNEL_TRACE", "0") == "1"
    res = run_bass_kernel_spmd(
        nc, in_maps, core_ids=list(range(N_CORES)), trace=trace
    )
    if trace:
        LAST_EXEC_NS = res.exec_time_ns

    out = np.zeros((Bsz, T, C), np.float32)
    for core in range(N_CORES):
        out[core // 4] += res.results[core]["outp"]
    return out
